# revision 20
# baseline (speedup 1.0000x reference)
"""Trainium2 Bass kernel for the GAU sparse-attention module.

Strategy: data-parallel over batch B=8, one sample per NeuronCore (8 cores).
Per core: the full [192,128,128] sample is kept resident in SBUF; two passes
over 32 tiles of 512 tokens (tokens ordered block-major within each tile so
each 128-token chunk is exactly 2 attention blocks).

  pass 0: instance-norm statistics (bn_stats/bn_aggr).
  pass 1: z = silu(xn@Wz1)@Wz2 -> k -> VQ argmin via PE scores + DVE max_index
          -> onehot; v' = [silu(xn@Wv) | 1]; accumulate deltaTv' = onehot^T @ v'.
          Stores z (PE-transposed, packed [128, 32, 8]) and delta indices.
  pass 2: rebuild q,k,onehot; qc = exp(q@cb^T); num|den = qc^T @ deltaTv'
          + per-block correction (exp(k@q^T) - onehot_s^T@qc) @ v';
          out = ((num/den)*g) @ Wc + x.

Precision: the k/z path (instance norm -> z1 -> z -> VQ scores) runs in fp32 so
the argmin matches the reference bit-for-bit in practice; the bulk value paths
(v, g, deltaTv, num/den, correction, Wc) run as float32r matmuls (fp32 with a
12-bit mantissa, 4x the fp32 matmul throughput at N>=256).
"""

import os
import sys

import numpy as np

sys.path.insert(0, "/opt/trn_rl_repo")

import concourse.bacc as bacc  # noqa: E402
import concourse.bass as bass  # noqa: E402
import concourse.mybir as mybir  # noqa: E402
import concourse.tile as tile  # noqa: E402
from concourse.tile_rust import add_dep_helper  # noqa: E402

F32 = mybir.dt.float32
F32R = mybir.dt.float32r
U32 = mybir.dt.uint32
AF = mybir.ActivationFunctionType
ALU = mybir.AluOpType

D, H, W = 192, 128, 128
DA, DB = 128, 64  # channel split across partitions
S = 64            # codebook size
VD = 384          # value dim (2*D)
NT = 32           # tiles of 512 tokens
SC = 2.0 ** (-0.25)
EPS = 1e-6

_last_results = None
_cached = None


def _build_nc():
    nc = bacc.Bacc("TRN2")
    # CoreSim doesn't implement the Silu LUT; GAU_SIM=1 builds with
    # sigmoid+multiply instead so the program can be checked in simulation.
    sim_compat = bool(os.environ.get("GAU_SIM"))

    def _silu(out, in_):
        if sim_compat:
            nc.scalar.activation(out=out, in_=in_, func=AF.Sigmoid)
            return nc.vector.tensor_tensor(out=out, in0=out, in1=in_, op=ALU.mult)
        return nc.scalar.activation(out=out, in_=in_, func=AF.Silu)

    x = nc.dram_tensor("x", [D, H, W], F32, kind="ExternalInput")
    wz1 = nc.dram_tensor("wz1", [D, D], F32R, kind="ExternalInput")  # pre-rounded
    wz2 = nc.dram_tensor("wz2", [D, 2], F32R, kind="ExternalInput")  # pre-rounded
    wvw = nc.dram_tensor("wvw", [D, VD], F32R, kind="ExternalInput")   # pre-rounded
    wgw = nc.dram_tensor("wgw", [D, VD], F32R, kind="ExternalInput")   # pre-rounded
    wcw = nc.dram_tensor("wcw", [VD, D], F32R, kind="ExternalInput")   # pre-rounded
    cbt = nc.dram_tensor("cbt", [2, S], F32R, kind="ExternalInput")    # pre-rounded
    cb3f = nc.dram_tensor("cb3f", [3, S], F32, kind="ExternalInput")
    qsc = nc.dram_tensor("qsc", [2, 1], F32, kind="ExternalInput")
    qbi = nc.dram_tensor("qbi", [2, 1], F32, kind="ExternalInput")
    ksc = nc.dram_tensor("ksc", [2, 1], F32, kind="ExternalInput")
    kbi = nc.dram_tensor("kbi", [2, 1], F32, kind="ExternalInput")
    idn = nc.dram_tensor("idn", [128, 128], F32, kind="ExternalInput")
    iot = nc.dram_tensor("iot", [128, S], F32, kind="ExternalInput")
    y = nc.dram_tensor("y", [D, H, W], F32, kind="ExternalOutput")
    vstash = nc.dram_tensor("vstash", [NT, 4, 128, VD + 2], F32R)

    with tile.TileContext(nc) as tc:
        from contextlib import ExitStack

        with ExitStack() as ctx:
            consts = ctx.enter_context(tc.tile_pool(name="consts", bufs=1))
            xpool = ctx.enter_context(tc.tile_pool(name="xpool", bufs=1))
            store = ctx.enter_context(tc.tile_pool(name="store", bufs=1))

            # ---- resident x ----
            xa = xpool.tile([DA, 16, 8, W], F32)
            xb = xpool.tile([DB, 16, 8, W], F32)
            for hi in range(16):
                nc.sync.dma_start(out=xa[:, hi], in_=x[0:DA, hi * 8:(hi + 1) * 8, :])
                nc.sync.dma_start(out=xb[:, hi], in_=x[DA:D, hi * 8:(hi + 1) * 8, :])

            # ---- weights / constants ----
            wz1a = consts.tile([DA, D], F32R)
            wz1b = consts.tile([DB, D], F32R)
            nc.sync.dma_start(out=wz1a, in_=wz1[0:DA, :])
            nc.sync.dma_start(out=wz1b, in_=wz1[DA:D, :])
            wz2a = consts.tile([DA, 2], F32R)
            wz2b = consts.tile([DB, 2], F32R)
            nc.sync.dma_start(out=wz2a, in_=wz2[0:DA, :])
            nc.sync.dma_start(out=wz2b, in_=wz2[DA:D, :])
            wva = consts.tile([DA, VD], F32R)
            wvb = consts.tile([DB, VD], F32R)
            nc.sync.dma_start(out=wva, in_=wvw[0:DA, :])
            nc.sync.dma_start(out=wvb, in_=wvw[DA:D, :])
            wga = consts.tile([DA, VD], F32R)
            wgb = consts.tile([DB, VD], F32R)
            nc.sync.dma_start(out=wga, in_=wgw[0:DA, :])
            nc.sync.dma_start(out=wgb, in_=wgw[DA:D, :])
            wc_t = consts.tile([128, 3, D], F32R)
            for vc in range(3):
                nc.sync.dma_start(out=wc_t[:, vc], in_=wcw[vc * 128:(vc + 1) * 128, :])
            cbt_t = consts.tile([2, S], F32R)
            nc.sync.dma_start(out=cbt_t, in_=cbt[:, :])
            # broadcast rows of cb3f ([2c0; 2c1; -|c|^2]) across partitions,
            # and gamma_k/beta_k as [128,1] per-partition scalars (DVE scores)
            cb3b = consts.tile([128, 3, S], F32)
            for r in range(3):
                srcap = cb3f[r:r + 1, :]
                nc.gpsimd.dma_start(
                    out=cb3b[:, r],
                    in_=bass.AP(tensor=srcap.tensor, offset=srcap.offset,
                                ap=[[0, 128]] + srcap.ap[1:]))
            kscb = consts.tile([128, 2], F32)
            kbib = consts.tile([128, 2], F32)
            for r in range(2):
                s1 = ksc[r:r + 1, 0:1]
                nc.gpsimd.dma_start(
                    out=kscb[:, r:r + 1],
                    in_=bass.AP(tensor=s1.tensor, offset=s1.offset,
                                ap=[[0, 128]] + s1.ap[1:]))
                s2 = kbi[r:r + 1, 0:1]
                nc.gpsimd.dma_start(
                    out=kbib[:, r:r + 1],
                    in_=bass.AP(tensor=s2.tensor, offset=s2.offset,
                                ap=[[0, 128]] + s2.ap[1:]))
            qsc_t = consts.tile([2, 1], F32)
            qbi_t = consts.tile([2, 1], F32)
            ksc_t = consts.tile([2, 1], F32)
            kbi_t = consts.tile([2, 1], F32)
            nc.sync.dma_start(out=qsc_t, in_=qsc[:, :])
            nc.sync.dma_start(out=qbi_t, in_=qbi[:, :])
            nc.sync.dma_start(out=ksc_t, in_=ksc[:, :])
            nc.sync.dma_start(out=kbi_t, in_=kbi[:, :])
            idn_t = consts.tile([128, 128], F32)
            nc.sync.dma_start(out=idn_t, in_=idn[:, :])
            idn_r = consts.tile([128, 128], F32R)
            nc.sync.dma_start(out=idn_r, in_=idn[:, :].bitcast(F32R))
            iot_t = consts.tile([128, S], F32)
            nc.sync.dma_start(out=iot_t, in_=iot[:, :])
            eps_a = consts.tile([DA, 1], F32)
            eps_b = consts.tile([DB, 1], F32)
            nc.vector.memset(eps_a, EPS)
            nc.vector.memset(eps_b, EPS)
            # block-diagonal diff holder: off-diagonal quadrants stay zero
            dfT2 = consts.tile([128, 128], F32R)
            nc.vector.memset(dfT2[:].bitcast(F32), 0.0)

            # ---- cross-pass stores ----
            ztst = store.tile([128, NT, 8], F32R)     # packed z^T per tile
            dlst = store.tile([128, NT, 4], F32)     # delta index (as f32) per chunk
            dtva = store.tile([S, VD + 2], F32)      # deltaTv' accumulator (fp32)
            dtvar = store.tile([S, VD + 2], F32R)    # rounded copy for pass 2
            nc.vector.memset(dtva, 0.0)

            # ---- instance norm stats ----
            sta = store.tile([DA, 32, 6], F32)
            stb = store.tile([DB, 32, 6], F32)
            xaf = xa[:].rearrange("p a b c -> p (a b c)")
            xbf = xb[:].rearrange("p a b c -> p (a b c)")
            for j in range(32):
                nc.vector.bn_stats(out=sta[:, j], in_=xaf[:, j * 512:(j + 1) * 512])
                nc.vector.bn_stats(out=stb[:, j], in_=xbf[:, j * 512:(j + 1) * 512])
            mva = store.tile([DA, 2], F32)
            mvb = store.tile([DB, 2], F32)
            nc.vector.bn_aggr(out=mva, in_=sta)
            nc.vector.bn_aggr(out=mvb, in_=stb)
            rsa = store.tile([DA, 1], F32)
            rsb = store.tile([DB, 1], F32)
            nc.scalar.activation(out=rsa, in_=mva[:, 1:2], func=AF.Sqrt, bias=eps_a)
            nc.scalar.activation(out=rsb, in_=mvb[:, 1:2], func=AF.Sqrt, bias=eps_b)
            nc.vector.reciprocal(out=rsa, in_=rsa)
            nc.vector.reciprocal(out=rsb, in_=rsb)
            mua = mva[:, 0:1]
            mub = mvb[:, 0:1]

            def make_xn(pool, t, dtype, tag):
                """Normalized tile [*, 512] in block-major token order."""
                hi, half = t // 2, t % 2
                xn_a = pool.tile([DA, 512], dtype, tag=tag + "a")
                xn_b = pool.tile([DB, 512], dtype, tag=tag + "b")
                # src (p, hs, w64) -> (p, j, hs, ws)
                sa = xa[:, hi, :, half * 64:(half + 1) * 64] \
                    .rearrange("p h (j w) -> p j h w", j=8)
                sb_ = xb[:, hi, :, half * 64:(half + 1) * 64] \
                    .rearrange("p h (j w) -> p j h w", j=8)
                da = xn_a[:].rearrange("p (j h w) -> p j h w", j=8, h=8)
                db = xn_b[:].rearrange("p (j h w) -> p j h w", j=8, h=8)
                nc.vector.tensor_scalar(da, sa, mua, rsa, ALU.subtract, ALU.mult)
                nc.vector.tensor_scalar(db, sb_, mub, rsb, ALU.subtract, ALU.mult)
                return xn_a, xn_b

            # ================= pass 1 =================
            with ExitStack() as p1:
                sb = p1.enter_context(tc.tile_pool(name="p1sb", bufs=3))
                pz1a = p1.enter_context(tc.tile_pool(name="pz1a", bufs=1, space="PSUM"))
                pz1b = p1.enter_context(tc.tile_pool(name="pz1b", bufs=1, space="PSUM"))
                pz = p1.enter_context(tc.tile_pool(name="pz", bufs=1, space="PSUM"))
                psm = p1.enter_context(tc.tile_pool(name="psm", bufs=2, space="PSUM"))
                pv = p1.enter_context(tc.tile_pool(name="pv", bufs=2, space="PSUM"))
                pdtv = p1.enter_context(tc.tile_pool(name="pdtv", bufs=1, space="PSUM"))

                for t in range(NT):
                    xr_a, xr_b = make_xn(sb, t, F32R, "xr")
                    z1a = pz1a.tile([DA, 512], F32)
                    z1b = pz1b.tile([DB, 512], F32)
                    nc.tensor.matmul(z1a, wz1a[:, 0:DA], xr_a, start=True, stop=False)
                    nc.tensor.matmul(z1a, wz1b[:, 0:DA], xr_b, start=False, stop=True)
                    nc.tensor.matmul(z1b, wz1a[:, DA:D], xr_a, start=True, stop=False)
                    nc.tensor.matmul(z1b, wz1b[:, DA:D], xr_b, start=False, stop=True)
                    z1sa = sb.tile([DA, 512], F32R, tag="z1sa")
                    z1sb = sb.tile([DB, 512], F32R, tag="z1sb")
                    _silu(z1sa[:], z1a[:])
                    _silu(z1sb[:], z1b[:])
                    zps = pz.tile([2, 512], F32)
                    nc.tensor.matmul(zps, wz2a, z1sa, start=True, stop=False)
                    nc.tensor.matmul(zps, wz2b, z1sb, start=False, stop=True)

                    # store z^T packed
                    z_sb = sb.tile([2, 512], F32R, tag="z_sb")
                    nc.scalar.copy(out=z_sb, in_=zps)
                    ztp = psm.tile([128, 8], F32R, tag="small")
                    for c in range(4):
                        nc.tensor.transpose(ztp[:, c * 2:(c + 1) * 2],
                                            z_sb[:, c * 128:(c + 1) * 128],
                                            idn_r[0:2, 0:2])
                    nc.scalar.copy(out=ztst[:, t, :], in_=ztp)

                    dtv = pdtv.tile([S, VD + 2], F32)
                    # kT[:, 2c]=k0, [:, 2c+1]=k1 per chunk, from the packed z^T
                    kT = sb.tile([128, 8], F32, tag="kT")
                    ztv = ztp[:].rearrange("p (c k) -> p c k", k=2)
                    ktv = kT[:].rearrange("p (c k) -> p c k", k=2)
                    nc.vector.tensor_scalar(ktv[:, :, 0], ztv[:, :, 0],
                                            kscb[:, 0:1], kbib[:, 0:1],
                                            ALU.mult, ALU.add)
                    nc.vector.tensor_scalar(ktv[:, :, 1], ztv[:, :, 1],
                                            kscb[:, 1:2], kbib[:, 1:2],
                                            ALU.mult, ALU.add)
                    for c in range(4):
                        # negated VQ score: 2c0*k0 + (2c1*k1 + (-|c|^2)) on DVE
                        scu = sb.tile([128, S], F32, tag="scu")
                        nc.vector.scalar_tensor_tensor(
                            out=scu, in0=cb3b[:, 1], scalar=kT[:, 2 * c + 1:2 * c + 2],
                            in1=cb3b[:, 2], op0=ALU.mult, op1=ALU.add)
                        scs = sb.tile([128, S], F32, tag="scs")
                        nc.vector.scalar_tensor_tensor(
                            out=scs, in0=cb3b[:, 0], scalar=kT[:, 2 * c:2 * c + 1],
                            in1=scu, op0=ALU.mult, op1=ALU.add)
                        vmx = sb.tile([128, 8], F32, tag="vmx")
                        nc.vector.max(out=vmx, in_=scs)
                        vix = sb.tile([128, 8], U32, tag="vix")
                        nc.vector.max_index(out=vix, in_max=vmx, in_values=scs)
                        nc.vector.tensor_copy(out=dlst[:, t, c:c + 1], in_=vix[:, 0:1])
                        oh = sb.tile([128, S], F32R, tag="oh")
                        nc.vector.tensor_scalar(oh, iot_t, dlst[:, t, c:c + 1], None,
                                                ALU.is_equal)
                        vps = pv.tile([128, VD], F32, tag="vps")
                        nc.tensor.matmul(vps, xr_a[:, c * 128:(c + 1) * 128], wva,
                                         start=True, stop=False)
                        nc.tensor.matmul(vps, xr_b[:, c * 128:(c + 1) * 128], wvb,
                                         start=False, stop=True)
                        vsb = sb.tile([128, VD + 2], F32R, tag="vsb")
                        _silu(vsb[:, 0:VD], vps[:])
                        nc.vector.memset(vsb[:, VD:VD + 1].bitcast(F32), 1.0)
                        nc.vector.memset(vsb[:, VD + 1:VD + 2].bitcast(F32), 0.0)
                        nc.tensor.matmul(dtv, oh, vsb, start=(c == 0), stop=(c == 3),
                                         skip_group_check=True)
                        nc.sync.dma_start(out=vstash[t, c], in_=vsb)
                    nc.vector.tensor_tensor(out=dtva, in0=dtva, in1=dtv, op=ALU.add)

            # ================= pass 2 =================
            with ExitStack() as p2:
                sb = p2.enter_context(tc.tile_pool(name="p2sb", bufs=2))
                vgp = p2.enter_context(tc.tile_pool(name="p2vg", bufs=4))
                psm = p2.enter_context(tc.tile_pool(name="p2sm", bufs=2, space="PSUM"))
                pqc = p2.enter_context(tc.tile_pool(name="pqc", bufs=1, space="PSUM"))
                pvg = p2.enter_context(tc.tile_pool(name="pvg", bufs=1, space="PSUM"))
                pnd = p2.enter_context(tc.tile_pool(name="pnd", bufs=2, space="PSUM"))
                po = p2.enter_context(tc.tile_pool(name="po", bufs=1, space="PSUM"))

                nc.vector.tensor_copy(out=dtvar, in_=dtva)

                prev_exp_last = None
                for t in range(NT):
                    hi, half = t // 2, t % 2
                    xr_a, xr_b = make_xn(sb, t, F32R, "xr2")

                    q_sb = sb.tile([2, 512], F32R, tag="q_sb")
                    k_sb = sb.tile([2, 512], F32R, tag="k_sb")
                    zre = psm.tile([2, 512], F32R, tag="small2")
                    for c in range(4):
                        nc.tensor.transpose(zre[:, c * 128:(c + 1) * 128],
                                            ztst[:, t, c * 2:(c + 1) * 2], idn_r)
                    nc.scalar.activation(out=q_sb, in_=zre, func=AF.Identity,
                                         bias=qbi_t, scale=qsc_t)
                    nc.scalar.activation(out=k_sb, in_=zre, func=AF.Identity,
                                         bias=kbi_t, scale=ksc_t)

                    # --- silu phase: g for all chunks; v loaded from stash ---
                    vsbs, gsbs = [], []
                    prev_silu = None
                    for c in range(4):
                        vsb = vgp.tile([128, VD + 2], F32R, tag="vsb2")
                        nc.sync.dma_start(out=vsb, in_=vstash[t, c])
                        gps = pvg.tile([128, VD], F32, tag="vg")
                        nc.tensor.matmul(gps, xr_a[:, c * 128:(c + 1) * 128], wga,
                                         start=True, stop=False)
                        nc.tensor.matmul(gps, xr_b[:, c * 128:(c + 1) * 128], wgb,
                                         start=False, stop=True)
                        gsb = vgp.tile([128, VD], F32, tag="gsb")
                        s2 = _silu(gsb[:], gps[:])
                        vsbs.append(vsb)
                        gsbs.append(gsb)
                        # chain silus so the scheduler can't interleave them
                        # with exp-table ops
                        if prev_silu is None:
                            if prev_exp_last is not None:
                                add_dep_helper(s2.ins, prev_exp_last.ins,
                                               sync=False,
                                               reason="act-table-phase")
                        else:
                            add_dep_helper(s2.ins, prev_silu.ins, sync=False,
                                           reason="act-table-chain")
                        prev_silu = s2
                    silu_last = prev_silu

                    # --- exp phase ---
                    qcp = pqc.tile([S, 512], F32)
                    nc.tensor.matmul(qcp, cbt_t, q_sb, start=True, stop=True)
                    qcs = sb.tile([S, 512], F32R, tag="qcs")
                    e0 = nc.scalar.activation(out=qcs, in_=qcp, func=AF.Exp)
                    add_dep_helper(e0.ins, silu_last.ins, sync=False,
                                   reason="act-table-phase")
                    exp_last = e0

                    ohs = sb.tile([S, 512], F32R, tag="ohs")
                    ohp = psm.tile([S, 512], F32R, tag="small2")
                    for c in range(4):
                        oh = sb.tile([128, S], F32R, tag="oh2")
                        nc.vector.tensor_scalar(oh, iot_t, dlst[:, t, c:c + 1], None,
                                                ALU.is_equal)
                        nc.tensor.transpose(ohp[:, c * 128:(c + 1) * 128], oh, idn_r)
                    nc.scalar.copy(out=ohs, in_=ohp)

                    wts_all = sb.tile([128, 3, 512], F32R, tag="wts_all")
                    for c in range(4):
                        vsb = vsbs[c]
                        gsb = gsbs[c]
                        nd = pnd.tile([128, VD + 2], F32)
                        nc.tensor.matmul(nd, qcs[:, c * 128:(c + 1) * 128], dtvar,
                                         start=True, stop=False, skip_group_check=True)
                        # aTe2 = k.q and aTh2 = onehot_s^T @ qc for the whole
                        # 128-token chunk in one MM each; only the two diagonal
                        # 64x64 blocks are meaningful and read below.
                        lo = c * 128
                        aTe2 = psm.tile([128, 128], F32, tag="small2")
                        aTh2 = psm.tile([128, 128], F32, tag="small2")
                        nc.tensor.matmul(aTe2, k_sb[:, lo:lo + 128],
                                         q_sb[:, lo:lo + 128],
                                         start=True, stop=True)
                        nc.tensor.matmul(aTh2, ohs[:, lo:lo + 128],
                                         qcs[:, lo:lo + 128],
                                         start=True, stop=True)
                        aTe = sb.tile([128, 128], F32, tag="aTe")
                        e1 = nc.scalar.activation(out=aTe, in_=aTe2, func=AF.Exp)
                        add_dep_helper(e1.ins, exp_last.ins, sync=False,
                                       reason="act-table-chain")
                        exp_last = e1
                        for b2 in range(2):
                            pr = slice(b2 * 64, (b2 + 1) * 64)
                            nc.vector.tensor_tensor(out=dfT2[pr, pr],
                                                    in0=aTe[pr, pr],
                                                    in1=aTh2[pr, pr],
                                                    op=ALU.subtract)
                        nc.tensor.matmul(nd, dfT2, vsb,
                                         start=False, stop=True,
                                         skip_group_check=True)

                        rden = sb.tile([128, 1], F32, tag="rden")
                        nc.vector.reciprocal(out=rden, in_=nd[:, VD:VD + 1])
                        wvg = sb.tile([128, VD], F32R, tag="wvg")
                        nc.vector.scalar_tensor_tensor(out=wvg, in0=nd[:, 0:VD],
                                                       scalar=rden, in1=gsb,
                                                       op0=ALU.mult, op1=ALU.mult)
                        for vc in range(3):
                            wtp = psm.tile([128, 128], F32R, tag="small2")
                            nc.tensor.transpose(
                                wtp, wvg[:, vc * 128:(vc + 1) * 128], idn_r)
                            nc.vector.tensor_copy(
                                out=wts_all[:, vc, c * 128:(c + 1) * 128], in_=wtp)

                    prev_exp_last = exp_last

                    # --- output projection (N=512 f32r matmuls) + residual ---
                    oap = po.tile([DA, 512], F32, tag="oa")
                    obp = po.tile([DB, 512], F32, tag="ob")
                    for vc in range(3):
                        nc.tensor.matmul(oap, wc_t[:, vc, 0:DA], wts_all[:, vc, :],
                                         start=(vc == 0), stop=(vc == 2),
                                         skip_group_check=True)
                        nc.tensor.matmul(obp, wc_t[:, vc, DA:D], wts_all[:, vc, :],
                                         start=(vc == 0), stop=(vc == 2),
                                         skip_group_check=True)
                    # psum cols are (j, hs, ws); spatial is (hs, j, ws)
                    oav = oap[:].rearrange("p (j h w) -> p h j w", j=8, h=8)
                    obv = obp[:].rearrange("p (j h w) -> p h j w", j=8, h=8)
                    xra = xa[:, hi, :, half * 64:(half + 1) * 64] \
                        .rearrange("p h (j w) -> p h j w", j=8)
                    xrb = xb[:, hi, :, half * 64:(half + 1) * 64] \
                        .rearrange("p h (j w) -> p h j w", j=8)
                    ya = sb.tile([DA, 8, 64], F32, tag="ya")
                    yb = sb.tile([DB, 8, 64], F32, tag="yb")
                    yav = ya[:].rearrange("p h (j w) -> p h j w", j=8)
                    ybv = yb[:].rearrange("p h (j w) -> p h j w", j=8)
                    nc.vector.tensor_tensor(out=yav, in0=oav, in1=xra, op=ALU.add)
                    nc.vector.tensor_tensor(out=ybv, in0=obv, in1=xrb, op=ALU.add)
                    nc.sync.dma_start(
                        out=y[0:DA, hi * 8:(hi + 1) * 8,
                              half * 64:(half + 1) * 64], in_=ya)
                    nc.sync.dma_start(
                        out=y[DA:D, hi * 8:(hi + 1) * 8,
                              half * 64:(half + 1) * 64], in_=yb)

    nc.compile()
    return nc


def _f32r_round(a):
    b = np.ascontiguousarray(a, np.float32).view(np.uint32).astype(np.uint64)
    b = (b + 0x400 + ((b >> 11) & 1)) & 0xFFFFF800
    return b.astype(np.uint32).view(np.float32)


def _host_consts(codebook, gamma_q, beta_q, gamma_k, beta_k):
    cb = np.asarray(codebook, np.float32)
    cbt = _f32r_round(np.ascontiguousarray(cb.T))          # [2, S]
    c2 = cb[:, 0] ** 2 + cb[:, 1] ** 2
    cb3f = np.stack([2.0 * cb[:, 0], 2.0 * cb[:, 1], -c2]).astype(np.float32)
    qsc = (np.asarray(gamma_q, np.float32) * SC).reshape(2, 1)
    qbi = (np.asarray(beta_q, np.float32) * SC).reshape(2, 1)
    ksc = (np.asarray(gamma_k, np.float32) * SC).reshape(2, 1)
    kbi = (np.asarray(beta_k, np.float32) * SC).reshape(2, 1)
    idn = np.eye(128, dtype=np.float32)
    iot = np.broadcast_to(np.arange(S, dtype=np.float32), (128, S)).copy()
    return dict(cbt=cbt, cb3f=cb3f, qsc=qsc, qbi=qbi, ksc=ksc, kbi=kbi,
                idn=idn, iot=iot)


def kernel(x, Wz1, Wz2, gamma_q, beta_q, gamma_k, beta_k, Wv, Wg, Wc, codebook):
    global _last_results, _cached
    from concourse.bass_utils import run_bass_kernel_spmd

    if _cached is None:
        _cached = _build_nc()
    nc = _cached

    consts = _host_consts(codebook, gamma_q, beta_q, gamma_k, beta_k)
    shared = dict(
        wz1=_f32r_round(Wz1), wz2=_f32r_round(Wz2),
        wvw=_f32r_round(Wv), wgw=_f32r_round(Wg),
        wcw=_f32r_round(Wc), **consts)
    x = np.asarray(x, np.float32)
    B = x.shape[0]
    in_maps = [dict(shared, x=np.ascontiguousarray(x[b])) for b in range(B)]

    trace = bool(os.environ.get("GAU_TRACE"))
    kwargs = {}
    if trace:
        kwargs = dict(trace=True, tmpdir=os.environ.get("GAU_TRACE_DIR") or None)
    res = run_bass_kernel_spmd(nc, in_maps, core_ids=list(range(B)), **kwargs)
    _last_results = res
    out = np.stack([res.results[b]["y"] for b in range(B)], axis=0)
    return out
